# revision 30
# baseline (speedup 1.0000x reference)
import os
import sys

sys.path.insert(0, "/opt/trn_rl_repo")

import numpy as np
from concourse import bass, mybir
from concourse.bass_utils import run_bass_kernel_spmd

# nn_PixelConv: feature (8,64,128,128) f32, kernel (8,36,128,128) f32
# -> out (8,64,256,256) f32.  out[n,c,2h+r,2w+q] =
#   sum_{dx,dy in 0..2} F[n,c,h+dy-1,w+dx-1] * K[n,(dx*3+dy)*4+r*2+q,h,w]
# Sharding: pure data-parallel, batch n -> core n.
#
# PE formulation: per pixel px, out[c, s] = sum_t patch[px, t, c] * K[px, t, s]
# (t = dx*3+dy, 9 taps; s = r*2+q, 4 subpixels).  Batch 14 pixels per matmul
# as a block-diagonal stationary:
#   lhsT[K=126=(i,t), M=56=(i,s)] = K-vals on the 14 diagonal 9x4 blocks
#   rhs [K=126,        N=64=c   ] = im2col patches (host-gathered)
#   out [M=56,         N=64    ]  = PSUM fp32, drained to f16 by DVE+Act.
#
# The stationary is 93% structural zeros, so only the 4 K-values per
# (pixel, tap) row are DMA'd (packed with the patches in one stream);
# the DVE expands them on device into the block-diagonal via one
# tensor_tensor against a constant 0/1 mask.
#
# Even groups land on PSUM partitions 0:56, odd groups on 56:112, so a
# 64-group chunk fills all 8 banks and the drains run 112 partitions
# wide (half the per-partition work of a 56-wide layout).
#
# DMA completion sems are per buffer slot: a single counting sem is
# ambiguous when several transfers are in flight (each incs 16 via
# independent per-engine sub-increments, so a threshold can be reached
# with an older transfer still incomplete).  Slot sems only ever carry
# increments from rounds <= the awaited one (issue order is gated on
# consumer progress), so their thresholds are exact.

N = 8
GPC = 14           # pixels per matmul group
TAPS = 9
KDIM = GPC * TAPS  # 126
MDIM = GPC * 4     # 56
CHUNK = 64         # groups per PSUM chunk (32 group-pairs on 112 partitions)
NCHUNK = 19
GROUPS = NCHUNK * CHUNK  # 1216; 1216*14 = 17024 >= 16384
JPC = CHUNK // 2   # 32 group-pairs per chunk
DVE_J = 10         # group-pairs per chunk drained by DVE (Act takes the rest)

LAST_EXEC_TIME_NS = None

f16 = mybir.dt.float16
f32 = mybir.dt.float32


def _build_program():
    nc = bass.Bass()
    # packed per-group input: 64 patch cols (c) + 4 K-values
    mk_ext = nc.dram_tensor("mk", [KDIM, NCHUNK, CHUNK, 68], f16, kind="ExternalInput")
    mask_ext = nc.dram_tensor("mask", [KDIM, GPC, 4], f16, kind="ExternalInput")
    o_ext = nc.dram_tensor("o", [120, NCHUNK, JPC, 64], f16, kind="ExternalOutput")

    import contextlib

    with contextlib.ExitStack() as stack:
        block = stack.enter_context(nc.Block())
        tsem = stack.enter_context(nc.semaphore("tsem"))
        vsem = stack.enter_context(nc.semaphore("vsem"))
        ssem = stack.enter_context(nc.semaphore("ssem"))
        bsem = stack.enter_context(nc.semaphore("bsem"))
        msem = stack.enter_context(nc.semaphore("msem"))
        osem = stack.enter_context(nc.semaphore("osem"))
        dsemb = [stack.enter_context(nc.semaphore(f"dsem{b}")) for b in range(3)]
        mk_sb = stack.enter_context(nc.sbuf_tensor([KDIM, 6, CHUNK, 68], f16))
        kb_sb = stack.enter_context(nc.sbuf_tensor([KDIM, 2, CHUNK, GPC, 4], f16))
        # output staged in SBUF, shipped at the end (keeps the DMA engines
        # on the input stream); split so AP offsets stay < 64KB
        OBC = [10, 9]
        ob_a = stack.enter_context(nc.sbuf_tensor([120, OBC[0] * JPC, 64], f16))
        ob_b = stack.enter_context(nc.sbuf_tensor([120, OBC[1] * JPC, 64], f16))

        def ob_slice(dc):
            if dc < OBC[0]:
                return ob_a[:, dc * JPC : (dc + 1) * JPC]
            return ob_b[:, (dc - OBC[0]) * JPC : (dc - OBC[0] + 1) * JPC]
        mask_sb = stack.enter_context(nc.sbuf_tensor([KDIM, GPC, 4], f16))
        warm_sb = stack.enter_context(nc.sbuf_tensor([120, 4], f16))
        ps = stack.enter_context(nc.psum_tensor([120, 2, JPC, 64], f32))

        @block.sync
        def _(sync):
            sync.dma_start(out=mask_sb[:], in_=mask_ext[:]).then_inc(msem, 16)
            # transfers 0,1 are single chunks (early PE start), then pairs,
            # into 6 rotating chunk-slots (3 transfer-slot sems)
            for k in range(11):
                if k < 2:
                    c0, c1 = k, k + 1
                else:
                    c0, c1 = 2 * k - 2, min(2 * k, NCHUNK)
                if k >= 3:
                    # sem k%3 reused from transfer k-3, chunk slots (k>=4)
                    # from transfers k-4/k-3; gate on those consumers
                    sync.wait_ge(tsem, max(1, 2 * k - 6))
                s0 = c0 % 6
                sync.dma_start(
                    out=mk_sb[:, s0 : s0 + (c1 - c0)], in_=mk_ext[:, c0:c1]
                ).then_inc(dsemb[k % 3], 16)
            # ob_a is complete once its 10 chunks are drained; flushing it
            # early overlaps ~5MB of output under the tail of the compute
            sync.wait_ge(ssem, OBC[0])
            sync.dma_start(
                out=o_ext[:, 0 : OBC[0]], in_=ob_a[:]
            ).then_inc(osem, 16)
            sync.wait_ge(ssem, NCHUNK)
            sync.dma_start(
                out=o_ext[:, OBC[0] : NCHUNK], in_=ob_b[:]
            ).then_inc(osem, 16)

        @block.vector
        def _(v):
            # rows 56:64 are never written by the matmuls but are read by
            # the wide drains; zero them once (drains on both engines are
            # ordered after this through bsem -> tsem)
            v.memset(ps[32:64], 0.0)
            # interleave block-diagonal builds (for PE) with PSUM drains
            v.wait_ge(msem, 16)
            for c in range(NCHUNK):
                if True:
                    # build chunk c: kb[p, g, j, s] = Kc[p, g, s] * mask[p, j, s]
                    k = c if c < 2 else c // 2 + 1
                    v.wait_ge(dsemb[k % 3], 16 * (k // 3 + 1))
                    if c >= 2:
                        v.wait_ge(tsem, c - 1)  # kb slot c%2 free
                    in1 = (
                        mk_sb[:, c % 6, :, 64:68]
                        .unsqueeze(2)
                        .broadcast_to([KDIM, CHUNK, GPC, 4])
                    )
                    in0 = (
                        mask_sb[:]
                        .unsqueeze(1)
                        .broadcast_to([KDIM, CHUNK, GPC, 4])
                    )
                    v.tensor_tensor(
                        out=kb_sb[:, c % 2], in0=in0, in1=in1,
                        op=mybir.AluOpType.mult,
                    ).then_inc(bsem, 1)


        @block.tensor
        def _(t):
            for c in range(NCHUNK):
                t.wait_ge(bsem, c + 1)
                k = c if c < 2 else c // 2 + 1
                t.wait_ge(dsemb[k % 3], 16 * (k // 3 + 1))
                if c >= 2:
                    # PSUM buffer c%2 reused -> drain of chunk c-2 done
                    t.wait_ge(ssem, c - 1)
                last = None
                for j in range(JPC):
                    # even group of the pair -> PSUM partitions 0:56,
                    # odd group -> 56:112
                    last = t.matmul(
                        ps[0:MDIM, c % 2, j],
                        kb_sb[:, c % 2, 2 * j],
                        mk_sb[:, c % 6, 2 * j, 0:64],
                        start=True, stop=True,
                    )
                    last = t.matmul(
                        ps[64:120, c % 2, j],
                        kb_sb[:, c % 2, 2 * j + 1],
                        mk_sb[:, c % 6, 2 * j + 1, 0:64],
                        start=True, stop=True,
                    )
                last.then_inc(tsem, 1)

        @block.scalar
        def _(s):
            # warm the activation table before the pipeline starts
            s.wait_ge(msem, 16)
            s.activation(
                out=warm_sb[:], in_=mask_sb[0:120, 0],
                func=mybir.ActivationFunctionType.Copy,
            )
            for c in range(NCHUNK):
                s.wait_ge(tsem, c + 1)
                s.activation(
                    out=ob_slice(c),
                    in_=ps[:, c % 2, :],
                    func=mybir.ActivationFunctionType.Copy,
                ).then_inc(ssem, 1)

    return nc


_NC = None
_HOOK_DONE = False
_IDX = None


def _install_ntff_hook():
    # bass_utils' trace path fetches the NTFF profile hook via
    # antenv.axon_hooks, which this image lacks. Install a shim and
    # register the ctypes-based hook (mirrors trn_boot.boot()).
    global _HOOK_DONE
    if _HOOK_DONE:
        return
    _HOOK_DONE = True
    try:
        import antenv.axon_hooks  # noqa: F401

        return
    except ImportError:
        pass
    try:
        import contextlib
        import ctypes
        import types

        import antenv

        mod = types.ModuleType("antenv.axon_hooks")
        holder = {"hook": None}
        mod.set_axon_ntff_profile_hook = lambda h: holder.__setitem__("hook", h)
        mod.get_axon_ntff_profile_hook = lambda: holder["hook"]
        sys.modules["antenv.axon_hooks"] = mod
        antenv.axon_hooks = mod

        lib = ctypes.CDLL("/opt/axon/libaxon_pjrt.so")
        if not hasattr(lib, "axon_start_nrt_profile"):
            return
        lib.axon_start_nrt_profile.argtypes = [
            ctypes.POINTER(ctypes.c_int64),
            ctypes.c_size_t,
        ]
        lib.axon_start_nrt_profile.restype = ctypes.c_int64
        lib.axon_stop_nrt_profile.argtypes = [ctypes.c_char_p]
        lib.axon_stop_nrt_profile.restype = ctypes.c_int64

        @contextlib.contextmanager
        def _hook(output_dir, device_ids):
            import jax

            jax.devices()
            if device_ids:
                ids = (ctypes.c_int64 * len(device_ids))(*device_ids)
                rc = lib.axon_start_nrt_profile(ids, len(device_ids))
            else:
                rc = lib.axon_start_nrt_profile(None, 0)
            if rc != 0:
                raise RuntimeError(f"axon_start_nrt_profile rc={rc}")
            try:
                yield
            finally:
                n = lib.axon_stop_nrt_profile(str(output_dir).encode())
                if n < 0:
                    raise RuntimeError(f"axon_stop_nrt_profile rc={n}")

        mod.set_axon_ntff_profile_hook(_hook)

        from concourse import bass_utils as _bu

        _bu.upload_artifacts = lambda tmpdir: "local://" + str(tmpdir)
    except Exception:
        pass


def _patch_index():
    # mv gather index [KDIM, GROUPS]: row (i,t) of group g reads padded-FT
    # linear row (h + t%3)*130 + (w + t//3) for pixel px = g*14+i.
    global _IDX
    if _IDX is not None:
        return _IDX
    px = np.arange(GROUPS * GPC)
    px = np.minimum(px, 16383)
    h, w = px // 128, px % 128
    t = np.arange(TAPS)
    dy, dx = t % 3, t // 3
    lin = (h[:, None] + dy[None, :]) * 130 + (w[:, None] + dx[None, :])
    idx = lin.reshape(GROUPS, GPC, TAPS).transpose(1, 2, 0).reshape(KDIM, GROUPS)
    _IDX = np.ascontiguousarray(idx)
    return _IDX


def _make_mask():
    m = np.zeros((GPC, TAPS, GPC, 4), np.float16)
    ii = np.arange(GPC)
    m[ii, :, ii, :] = 1.0
    return m.reshape(KDIM, GPC, 4)


def _prep_core_inputs(feat_n, kern_n):
    # feat_n (64,128,128) f32, kern_n (36,128,128) f32
    idx = _patch_index()
    ftp = np.zeros((130, 130, 64), np.float16)
    ftp[1:129, 1:129] = feat_n.transpose(1, 2, 0)
    mk = np.empty((KDIM, GROUPS, 68), np.float16)
    mk[:, :, 0:64] = ftp.reshape(16900, 64)[idx]

    kres = kern_n.reshape(TAPS, 4, 16384).astype(np.float16)
    kpad = np.zeros((TAPS, 4, GROUPS * GPC), np.float16)
    kpad[:, :, :16384] = kres
    # Kc[(i,t), g, s]
    mk[:, :, 64:68] = (
        kpad.reshape(TAPS, 4, GROUPS, GPC)
        .transpose(3, 0, 2, 1)
        .reshape(KDIM, GROUPS, 4)
    )
    return {
        "mk": np.ascontiguousarray(mk).reshape(KDIM, NCHUNK, CHUNK, 68),
        "mask": _make_mask(),
    }


def _assemble_output(raw_o):
    # raw_o (120, NCHUNK, JPC, 64) f16: even group of pair j on partitions
    # 0:56, odd group on 64:120 (56:64 is junk from the wide drain)
    o = np.stack([raw_o[0:MDIM], raw_o[64:120]])  # (2, 56, NCHUNK, JPC, 64)
    o = o.astype(np.float32).reshape(2, GPC, 4, NCHUNK, JPC, 64)
    # -> [c, chunk, j, h2, i, s] so (chunk, j, h2) composes the group index
    o = o.transpose(5, 3, 4, 0, 1, 2).reshape(64, GROUPS * GPC, 4)[:, :16384]
    o = o.reshape(64, 128, 128, 2, 2).transpose(0, 1, 3, 2, 4)
    return o.reshape(64, 256, 256)


def kernel(feature: np.ndarray, kernel: np.ndarray) -> np.ndarray:
    global _NC, LAST_EXEC_TIME_NS
    if _NC is None:
        _NC = _build_program()
    feature = np.asarray(feature, dtype=np.float32)
    kernel = np.asarray(kernel, dtype=np.float32)
    in_maps = [_prep_core_inputs(feature[n], kernel[n]) for n in range(N)]
    trace = os.environ.get("PIXELCONV_TRACE", "") not in ("", "0")
    if trace:
        _install_ntff_hook()
    res = run_bass_kernel_spmd(
        _NC, in_maps, core_ids=list(range(N)), trace=trace
    )
    LAST_EXEC_TIME_NS = getattr(res, "exec_time_ns", None)
    out = np.stack([_assemble_output(res.results[n]["o"]) for n in range(N)])
    return out.astype(np.float32)


# revision 33
# speedup vs baseline: 1.1063x; 1.1063x over previous
import os
import sys

sys.path.insert(0, "/opt/trn_rl_repo")

import numpy as np
from concourse import bass, mybir
from concourse.bass_utils import run_bass_kernel_spmd

# nn_PixelConv: feature (8,64,128,128) f32, kernel (8,36,128,128) f32
# -> out (8,64,256,256) f32.  out[n,c,2h+r,2w+q] =
#   sum_{dx,dy in 0..2} F[n,c,h+dy-1,w+dx-1] * K[n,(dx*3+dy)*4+r*2+q,h,w]
# Sharding: pure data-parallel, batch n -> core n.
#
# PE formulation: per pixel px, out[c, s] = sum_t patch[px, t, c] * K[px, t, s]
# (t = dx*3+dy, 9 taps; s = r*2+q, 4 subpixels).  Batch 14 pixels per matmul
# as a block-diagonal stationary:
#   lhsT[K=126=(i,t), M=56=(i,s)] = K-vals on the 14 diagonal 9x4 blocks
#   rhs [K=126,        N=64=c   ] = im2col patches (host-gathered)
#   out [M=56,         N=64    ]  = PSUM fp32, drained to f16 by DVE+Act.
#
# The stationary is 93% structural zeros, so only the 4 K-values per
# (pixel, tap) row are DMA'd (packed with the patches in one stream);
# the DVE expands them on device into the block-diagonal via one
# tensor_tensor against a constant 0/1 mask.
#
# Even groups land on PSUM partitions 0:56, odd groups on 56:112, so a
# 64-group chunk fills all 8 banks and the drains run 112 partitions
# wide (half the per-partition work of a 56-wide layout).
#
# DMA completion sems are per buffer slot: a single counting sem is
# ambiguous when several transfers are in flight (each incs 16 via
# independent per-engine sub-increments, so a threshold can be reached
# with an older transfer still incomplete).  Slot sems only ever carry
# increments from rounds <= the awaited one (issue order is gated on
# consumer progress), so their thresholds are exact.

N = 8
GPC = 14           # pixels per matmul group
TAPS = 9
KDIM = GPC * TAPS  # 126
MDIM = GPC * 4     # 56
CHUNK = 64         # groups per PSUM chunk (32 group-pairs on 112 partitions)
NCHUNK = 19
JPC = CHUNK // 2   # 32 group-pairs per chunk
JL = 16            # group-pairs in the final half-size chunk
TOTJ = (NCHUNK - 1) * JPC + JL  # 592
GROUPS = 2 * TOTJ  # 1184; 1184*14 = 16576 >= 16384
DVE_J = 10         # group-pairs per chunk drained by DVE (Act takes the rest)

LAST_EXEC_TIME_NS = None

f16 = mybir.dt.float16
f32 = mybir.dt.float32


def _build_program():
    nc = bass.Bass()
    # packed per-group input: 64 patch cols (c) + 4 K-values
    mk_ext = nc.dram_tensor("mk", [KDIM, GROUPS, 68], f16, kind="ExternalInput")
    mask_ext = nc.dram_tensor("mask", [KDIM, GPC, 4], f16, kind="ExternalInput")
    o_ext = nc.dram_tensor("o", [120, TOTJ, 64], f16, kind="ExternalOutput")

    import contextlib

    with contextlib.ExitStack() as stack:
        block = stack.enter_context(nc.Block())
        tsem = stack.enter_context(nc.semaphore("tsem"))
        vsem = stack.enter_context(nc.semaphore("vsem"))
        ssem = stack.enter_context(nc.semaphore("ssem"))
        bsem = stack.enter_context(nc.semaphore("bsem"))
        msem = stack.enter_context(nc.semaphore("msem"))
        osem = stack.enter_context(nc.semaphore("osem"))
        dsemb = [stack.enter_context(nc.semaphore(f"dsem{b}")) for b in range(3)]
        mk_sb = stack.enter_context(nc.sbuf_tensor([KDIM, 6 * CHUNK, 68], f16))
        kb_sb = stack.enter_context(nc.sbuf_tensor([KDIM, 2, CHUNK, GPC, 4], f16))
        # output staged in SBUF, shipped at the end (keeps the DMA engines
        # on the input stream); split so AP offsets stay < 64KB
        OBC = [10, 9]
        OBJ = [10 * JPC, 8 * JPC + JL]  # j-slots per staging tensor
        ob_a = stack.enter_context(nc.sbuf_tensor([120, OBJ[0], 64], f16))
        ob_b = stack.enter_context(nc.sbuf_tensor([120, OBJ[1], 64], f16))

        def _njp(dc):
            return JL if dc == NCHUNK - 1 else JPC

        def ob_slice(dc):
            if dc < OBC[0]:
                return ob_a[:, dc * JPC : dc * JPC + _njp(dc)]
            j0 = (dc - OBC[0]) * JPC
            return ob_b[:, j0 : j0 + _njp(dc)]
        mask_sb = stack.enter_context(nc.sbuf_tensor([KDIM, GPC, 4], f16))
        warm_sb = stack.enter_context(nc.sbuf_tensor([120, 4], f16))
        ps = stack.enter_context(nc.psum_tensor([120, 2, JPC, 64], f32))

        @block.sync
        def _(sync):
            sync.dma_start(out=mask_sb[:], in_=mask_ext[:]).then_inc(msem, 16)
            # transfers 0,1 are single chunks (early PE start), then pairs,
            # into 6 rotating chunk-slots (3 transfer-slot sems)
            for k in range(11):
                if k < 2:
                    c0, c1 = k, k + 1
                else:
                    c0, c1 = 2 * k - 2, min(2 * k, NCHUNK)
                if k >= 3:
                    # sem k%3 reused from transfer k-3, chunk slots (k>=4)
                    # from transfers k-4/k-3; gate on those consumers
                    sync.wait_ge(tsem, max(1, 2 * k - 6))
                s0 = c0 % 6
                g0 = c0 * CHUNK
                g1 = min(c1 * CHUNK, GROUPS)
                sync.dma_start(
                    out=mk_sb[:, s0 * CHUNK : s0 * CHUNK + (g1 - g0)],
                    in_=mk_ext[:, g0:g1],
                ).then_inc(dsemb[k % 3], 16)
            sync.wait_ge(ssem, NCHUNK)
            sync.dma_start(
                out=o_ext[:, 0 : OBJ[0]], in_=ob_a[:]
            ).then_inc(osem, 16)
            sync.dma_start(
                out=o_ext[:, OBJ[0] : TOTJ], in_=ob_b[:]
            ).then_inc(osem, 16)

        @block.vector
        def _(v):
            # rows 56:64 are never written by the matmuls but are read by
            # the wide drains; zero them once (drains on both engines are
            # ordered after this through bsem -> tsem)
            v.memset(ps[32:64], 0.0)
            # interleave block-diagonal builds (for PE) with PSUM drains
            v.wait_ge(msem, 16)
            for c in range(NCHUNK):
                if True:
                    # build chunk c: kb[p, g, j, s] = Kc[p, g, s] * mask[p, j, s]
                    k = c if c < 2 else c // 2 + 1
                    v.wait_ge(dsemb[k % 3], 16 * (k // 3 + 1))
                    if c >= 2:
                        v.wait_ge(tsem, c - 1)  # kb slot c%2 free
                    ng = 2 * (JL if c == NCHUNK - 1 else JPC)
                    in1 = (
                        mk_sb[:, (c % 6) * CHUNK : (c % 6) * CHUNK + ng, 64:68]
                        .unsqueeze(2)
                        .broadcast_to([KDIM, ng, GPC, 4])
                    )
                    in0 = (
                        mask_sb[:]
                        .unsqueeze(1)
                        .broadcast_to([KDIM, ng, GPC, 4])
                    )
                    v.tensor_tensor(
                        out=kb_sb[:, c % 2, 0:ng], in0=in0, in1=in1,
                        op=mybir.AluOpType.mult,
                    ).then_inc(bsem, 1)


        @block.tensor
        def _(t):
            for c in range(NCHUNK):
                t.wait_ge(bsem, c + 1)
                k = c if c < 2 else c // 2 + 1
                t.wait_ge(dsemb[k % 3], 16 * (k // 3 + 1))
                if c >= 2:
                    # PSUM buffer c%2 reused -> drain of chunk c-2 done
                    t.wait_ge(ssem, c - 1)
                last = None
                for j in range(JL if c == NCHUNK - 1 else JPC):
                    # even group of the pair -> PSUM partitions 0:56,
                    # odd group -> 56:112
                    last = t.matmul(
                        ps[0:MDIM, c % 2, j],
                        kb_sb[:, c % 2, 2 * j],
                        mk_sb[:, (c % 6) * CHUNK + 2 * j, 0:64],
                        start=True, stop=True,
                    )
                    last = t.matmul(
                        ps[64:120, c % 2, j],
                        kb_sb[:, c % 2, 2 * j + 1],
                        mk_sb[:, (c % 6) * CHUNK + 2 * j + 1, 0:64],
                        start=True, stop=True,
                    )
                last.then_inc(tsem, 1)

        @block.scalar
        def _(s):
            # warm the activation table before the pipeline starts
            s.wait_ge(msem, 16)
            s.activation(
                out=warm_sb[:], in_=mask_sb[0:120, 0],
                func=mybir.ActivationFunctionType.Copy,
            )
            for c in range(NCHUNK):
                s.wait_ge(tsem, c + 1)
                s.activation(
                    out=ob_slice(c),
                    in_=ps[:, c % 2, 0 : (JL if c == NCHUNK - 1 else JPC)],
                    func=mybir.ActivationFunctionType.Copy,
                ).then_inc(ssem, 1)

    return nc


_NC = None
_HOOK_DONE = False
_IDX = None


def _install_ntff_hook():
    # bass_utils' trace path fetches the NTFF profile hook via
    # antenv.axon_hooks, which this image lacks. Install a shim and
    # register the ctypes-based hook (mirrors trn_boot.boot()).
    global _HOOK_DONE
    if _HOOK_DONE:
        return
    _HOOK_DONE = True
    try:
        import antenv.axon_hooks  # noqa: F401

        return
    except ImportError:
        pass
    try:
        import contextlib
        import ctypes
        import types

        import antenv

        mod = types.ModuleType("antenv.axon_hooks")
        holder = {"hook": None}
        mod.set_axon_ntff_profile_hook = lambda h: holder.__setitem__("hook", h)
        mod.get_axon_ntff_profile_hook = lambda: holder["hook"]
        sys.modules["antenv.axon_hooks"] = mod
        antenv.axon_hooks = mod

        lib = ctypes.CDLL("/opt/axon/libaxon_pjrt.so")
        if not hasattr(lib, "axon_start_nrt_profile"):
            return
        lib.axon_start_nrt_profile.argtypes = [
            ctypes.POINTER(ctypes.c_int64),
            ctypes.c_size_t,
        ]
        lib.axon_start_nrt_profile.restype = ctypes.c_int64
        lib.axon_stop_nrt_profile.argtypes = [ctypes.c_char_p]
        lib.axon_stop_nrt_profile.restype = ctypes.c_int64

        @contextlib.contextmanager
        def _hook(output_dir, device_ids):
            import jax

            jax.devices()
            if device_ids:
                ids = (ctypes.c_int64 * len(device_ids))(*device_ids)
                rc = lib.axon_start_nrt_profile(ids, len(device_ids))
            else:
                rc = lib.axon_start_nrt_profile(None, 0)
            if rc != 0:
                raise RuntimeError(f"axon_start_nrt_profile rc={rc}")
            try:
                yield
            finally:
                n = lib.axon_stop_nrt_profile(str(output_dir).encode())
                if n < 0:
                    raise RuntimeError(f"axon_stop_nrt_profile rc={n}")

        mod.set_axon_ntff_profile_hook(_hook)

        from concourse import bass_utils as _bu

        _bu.upload_artifacts = lambda tmpdir: "local://" + str(tmpdir)
    except Exception:
        pass


def _patch_index():
    # mv gather index [KDIM, GROUPS]: row (i,t) of group g reads padded-FT
    # linear row (h + t%3)*130 + (w + t//3) for pixel px = g*14+i.
    global _IDX
    if _IDX is not None:
        return _IDX
    px = np.arange(GROUPS * GPC)
    px = np.minimum(px, 16383)
    h, w = px // 128, px % 128
    t = np.arange(TAPS)
    dy, dx = t % 3, t // 3
    lin = (h[:, None] + dy[None, :]) * 130 + (w[:, None] + dx[None, :])
    idx = lin.reshape(GROUPS, GPC, TAPS).transpose(1, 2, 0).reshape(KDIM, GROUPS)
    _IDX = np.ascontiguousarray(idx)
    return _IDX


def _make_mask():
    m = np.zeros((GPC, TAPS, GPC, 4), np.float16)
    ii = np.arange(GPC)
    m[ii, :, ii, :] = 1.0
    return m.reshape(KDIM, GPC, 4)


def _prep_core_inputs(feat_n, kern_n):
    # feat_n (64,128,128) f32, kern_n (36,128,128) f32
    idx = _patch_index()
    ftp = np.zeros((130, 130, 64), np.float16)
    ftp[1:129, 1:129] = feat_n.transpose(1, 2, 0)
    mk = np.empty((KDIM, GROUPS, 68), np.float16)
    mk[:, :, 0:64] = ftp.reshape(16900, 64)[idx]

    kres = kern_n.reshape(TAPS, 4, 16384).astype(np.float16)
    kpad = np.zeros((TAPS, 4, GROUPS * GPC), np.float16)
    kpad[:, :, :16384] = kres
    # Kc[(i,t), g, s]
    mk[:, :, 64:68] = (
        kpad.reshape(TAPS, 4, GROUPS, GPC)
        .transpose(3, 0, 2, 1)
        .reshape(KDIM, GROUPS, 4)
    )
    return {
        "mk": np.ascontiguousarray(mk),
        "mask": _make_mask(),
    }


def _assemble_output(raw_o):
    # raw_o (120, TOTJ, 64) f16: even group of pair j on partitions 0:56,
    # odd group on 64:120 (56:64 is junk from the wide drain)
    o = np.stack([raw_o[0:MDIM], raw_o[64:120]])  # (2, 56, TOTJ, 64)
    o = o.astype(np.float32).reshape(2, GPC, 4, TOTJ, 64)
    # -> [c, j, h2, i, s] so (j, h2) composes the group index
    o = o.transpose(4, 3, 0, 1, 2).reshape(64, GROUPS * GPC, 4)[:, :16384]
    o = o.reshape(64, 128, 128, 2, 2).transpose(0, 1, 3, 2, 4)
    return o.reshape(64, 256, 256)


def kernel(feature: np.ndarray, kernel: np.ndarray) -> np.ndarray:
    global _NC, LAST_EXEC_TIME_NS
    if _NC is None:
        _NC = _build_program()
    feature = np.asarray(feature, dtype=np.float32)
    kernel = np.asarray(kernel, dtype=np.float32)
    in_maps = [_prep_core_inputs(feature[n], kernel[n]) for n in range(N)]
    trace = os.environ.get("PIXELCONV_TRACE", "") not in ("", "0")
    if trace:
        _install_ntff_hook()
    res = run_bass_kernel_spmd(
        _NC, in_maps, core_ids=list(range(N)), trace=trace
    )
    LAST_EXEC_TIME_NS = getattr(res, "exec_time_ns", None)
    out = np.stack([_assemble_output(res.results[n]["o"]) for n in range(N)])
    return out.astype(np.float32)
